# revision 8
# baseline (speedup 1.0000x reference)
"""Multi-head self-attention (RoPE, causal) Trainium2 Bass kernel.

Sharding: 8 cores = 2 batches x 4 head-groups (4 heads each).
Each core computes QKV projections for its heads (feature-major via x^T),
RoPE, causal attention with transposed scores (softmax along partitions
handled via exp + ones-column denominator in the V matmul), and a partial
output projection over its head slice. The host sums the 4 partials per
batch (reduce step of the tensor-parallel output projection).

All matmuls run in fp32r (fp32 data, high-half weights) at full PE rate.
"""
import math
import os
from contextlib import ExitStack

import numpy as np

import concourse.bass as bass
import concourse.tile as tile
from concourse import bacc, mybir

F32 = mybir.dt.float32
F32R = mybir.dt.float32r
EXP = mybir.ActivationFunctionType.Exp

B, S, D, H, DH = 2, 2048, 1024, 16, 64
THETA = 10000.0
CORES = 8
HPC = 4                    # heads per core
F = HPC * DH               # 256 features per core
SCALE = 1.0 / math.sqrt(DH)
NKT = D // 128             # 8 k tiles
NSB = S // 512             # 4 seq blocks of 512
NST = S // 128             # 16 seq tiles of 128

_CACHED = {}


def _build_program(use_rs: bool):
    nc = bacc.Bacc("TRN2", target_bir_lowering=False, debug=False,
                   num_devices=CORES)

    xT = nc.dram_tensor("xT", [D, S], F32R, kind="ExternalInput")
    wqT = nc.dram_tensor("wqT", [D, F], F32R, kind="ExternalInput")
    wkT = nc.dram_tensor("wkT", [D, F], F32R, kind="ExternalInput")
    wvT = nc.dram_tensor("wvT", [D, F], F32R, kind="ExternalInput")
    woT = nc.dram_tensor("woT", [F, D], F32R, kind="ExternalInput")
    ropeA_d = nc.dram_tensor("ropeA", [128, S], F32, kind="ExternalInput")
    ropeB2_d = nc.dram_tensor("ropeB2", [128, S], F32, kind="ExternalInput")
    maskP_d = nc.dram_tensor("maskP", [2 * 128, 1024], F32R, kind="ExternalInput")

    if use_rs:
        out_d = nc.dram_tensor("out_rs", [S // 4, D], F32, kind="ExternalOutput")
        cc_ins = [nc.dram_tensor(f"cc_in{j}", [512, D], F32) for j in range(NSB)]
        cc_outs = [nc.dram_tensor(f"cc_out{j}", [128, D], F32) for j in range(NSB)]
        groups = [[0, 1, 2, 3], [4, 5, 6, 7]]
    else:
        out_d = nc.dram_tensor("partial", [S, D], F32, kind="ExternalOutput")

    den_dram = nc.dram_tensor("den_dram", [NSB * H // 2, 512], F32)  # scratch

    with tile.TileContext(nc) as tc, ExitStack() as ctx:
        persist = ctx.enter_context(tc.tile_pool(name="persist", bufs=1))

        # ---- persistent tiles ----
        wq_t = [persist.tile([128, F], F32R, tag=f"wq{k}", name=f"wq{k}") for k in range(NKT)]
        wk_t = [persist.tile([128, F], F32R, tag=f"wk{k}", name=f"wk{k}") for k in range(NKT)]
        wv_t = [persist.tile([128, F], F32R, tag=f"wv{k}", name=f"wv{k}") for k in range(NKT)]
        wo_t = [persist.tile([128, D], F32R, tag=f"wo{t}", name=f"wo{t}") for t in range(2)]
        ropeA = persist.tile([128, S], F32, tag="ropeA", name="ropeA")
        ropeB2 = persist.tile([128, S], F32, tag="ropeB2", name="ropeB2")
        maskP = [persist.tile([128, 1024], F32R, tag=f"maskP{i}", name=f"maskP{i}") for i in range(2)]
        QT = [persist.tile([128, S], F32R, tag=f"QT{t}", name=f"QT{t}") for t in range(2)]
        KT = [persist.tile([128, S], F32R, tag=f"KT{t}", name=f"KT{t}") for t in range(2)]
        Vaug = [persist.tile([128, 260], F32R, tag=f"Vaug{st}", name=f"Vaug{st}") for st in range(NST)]
        attT = [persist.tile([128, S], F32R, tag=f"attT{t}", name=f"attT{t}") for t in range(2)]
        ones4 = persist.tile([128, 4], F32, tag="ones4", name="ones4")
        nc.vector.memset(ones4, 1.0)

        for k in range(NKT):
            nc.sync.dma_start(out=wq_t[k], in_=wqT[128 * k:128 * (k + 1), :])
            nc.sync.dma_start(out=wk_t[k], in_=wkT[128 * k:128 * (k + 1), :])
            nc.sync.dma_start(out=wv_t[k], in_=wvT[128 * k:128 * (k + 1), :])
        for t in range(2):
            nc.sync.dma_start(out=wo_t[t], in_=woT[128 * t:128 * (t + 1), :])
        nc.sync.dma_start(out=ropeA, in_=ropeA_d[:, :])
        nc.sync.dma_start(out=ropeB2, in_=ropeB2_d[:, :])
        for i in range(2):
            nc.sync.dma_start(out=maskP[i], in_=maskP_d[128 * i:128 * (i + 1), :])

        # ---- Phase 1: Q/K projections + RoPE (feature-major) ----
        # k-outer passes; x^T streamed through a small pool. 8 psum
        # accumulators (2 feature-quadrants x 4 seq blocks) per pass.
        with tc.tile_pool(name="xpool", bufs=3) as xpool, \
             tc.tile_pool(name="ptmp", bufs=2) as ptmp, \
             tc.tile_pool(name="psProj", bufs=8, space="PSUM") as psProj:
            for (w_t, dest) in ((wq_t, QT), (wk_t, KT)):
                ps = [psProj.tile([128, 512], F32, tag="proj", name="proj")
                      for _ in range(8)]
                for k in range(NKT):
                    xk = xpool.tile([128, S], F32R, tag="xt", name="xt")
                    nc.sync.dma_start(out=xk, in_=xT[128 * k:128 * (k + 1), :])
                    for t in range(2):
                        fs = slice(128 * t, 128 * (t + 1))
                        for sb in range(NSB):
                            cs = slice(512 * sb, 512 * (sb + 1))
                            nc.tensor.matmul(ps[4 * t + sb], w_t[k][:, fs],
                                             xk[:, cs],
                                             start=(k == 0), stop=(k == NKT - 1))
                for t in range(2):
                    for sb in range(NSB):
                        cs = slice(512 * sb, 512 * (sb + 1))
                        p_ = ps[4 * t + sb]
                        t1 = ptmp.tile([128, 512], F32, tag="t1", name="t1")
                        nc.vector.tensor_mul(t1, p_, ropeA[:, cs])
                        t2p = ptmp.tile([128, 512], F32, tag="t2p", name="t2p")
                        nc.vector.tensor_mul(t2p, p_, ropeB2[:, cs])
                        t2 = ptmp.tile([128, 512], F32, tag="t2", name="t2")
                        for blk in range(4):
                            a, b = 32 * blk, 32 * (blk ^ 1)
                            nc.sync.dma_start(out=t2[a:a + 32, :],
                                              in_=t2p[b:b + 32, :])
                        nc.vector.tensor_add(dest[t][:, cs], t1, t2)

        # ---- Phase 2: V projection into Vaug (seq-major, ones cols) ----
        # k-inner per s-tile with sliced x loads; psum bufs=2.
        with tc.tile_pool(name="xvpool", bufs=8) as xvpool, \
             tc.tile_pool(name="psV", bufs=2, space="PSUM") as psV:
            for st in range(NST):
                ss = slice(128 * st, 128 * (st + 1))
                ps = psV.tile([128, 256], F32, tag="projv", name="projv")
                for k in range(NKT):
                    xs_ = xvpool.tile([128, 128], F32R, tag="xs", name="xs")
                    nc.sync.dma_start(out=xs_,
                                      in_=xT[128 * k:128 * (k + 1), ss])
                    nc.tensor.matmul(ps, xs_, wv_t[k],
                                     start=(k == 0), stop=(k == NKT - 1))
                dst = Vaug[st][:, 0:260].rearrange("p (h c) -> p h c", h=HPC)
                nc.vector.tensor_copy(dst[:, :, 0:64],
                                      ps.rearrange("p (h c) -> p h c", h=HPC))
                nc.vector.tensor_copy(Vaug[st][:, 64:260:65], ones4)

        # ---- Phase 3+4: attention per seq block, then partial Wo ----
        with tc.tile_pool(name="psA", bufs=2, space="PSUM") as psA, \
             tc.tile_pool(name="psB", bufs=2, space="PSUM") as psB, \
             tc.tile_pool(name="epool", bufs=3) as epool, \
             tc.tile_pool(name="ntmp", bufs=4) as ntmp, \
             tc.tile_pool(name="opool", bufs=3) as opool:
            for j in range(NSB):
                qs = slice(512 * j, 512 * (j + 1))
                n_pair = 2 * (j + 1)
                for hp in range(2):
                    t = hp
                    pn = [psB.tile([65, 512], F32, tag="pn", name="pn") for _ in range(2)]
                    for p in range(n_pair):
                        ska, skb = 2 * p, 2 * p + 1
                        psS = [psA.tile([128, 1024], F32, tag="score", name="score")
                               for _ in range(2)]
                        for (sk, half) in ((ska, slice(0, 512)),
                                           (skb, slice(512, 1024))):
                            ks = slice(128 * sk, 128 * (sk + 1))
                            for hh in range(2):
                                rs = slice(64 * hh, 64 * (hh + 1))
                                nc.tensor.matmul(psS[hh][:, half],
                                                 KT[t][rs, ks], QT[t][rs, qs],
                                                 start=True, stop=True)
                        E = [epool.tile([128, 1024], F32R, tag="E", name="E")
                             for _ in range(2)]
                        for hh in range(2):
                            nc.scalar.activation(out=E[hh], in_=psS[hh],
                                                 func=EXP, scale=SCALE)
                        if p >= n_pair - 2:
                            m = maskP[p - (n_pair - 2)]
                            for hh in range(2):
                                nc.vector.tensor_mul(E[hh], E[hh], m)
                        for hh in range(2):
                            h = 2 * hp + hh
                            vc = slice(65 * (h % HPC), 65 * (h % HPC) + 65)
                            nc.tensor.matmul(pn[hh], Vaug[ska][:, vc],
                                             E[hh][:, 0:512],
                                             start=(p == 0), stop=False)
                            nc.tensor.matmul(pn[hh], Vaug[skb][:, vc],
                                             E[hh][:, 512:1024],
                                             start=False, stop=(p == n_pair - 1))
                    # normalize -> attT
                    for hh in range(2):
                        h = 2 * hp + hh
                        rc = ntmp.tile([1, 512], F32, tag="rc", name="rc")
                        nc.vector.reciprocal(rc, pn[hh][64:65, :])
                        drow = j * 4 + hp * 2 + hh
                        nc.sync.dma_start(out=den_dram[drow:drow + 1, :], in_=rc)
                        bc = ntmp.tile([64, 512], F32, tag="bc", name="bc")
                        nc.sync.dma_start(
                            out=bc,
                            in_=den_dram[drow:drow + 1, :].to_broadcast([64, 512]))
                        rs = slice(64 * hh, 64 * (hh + 1))
                        nc.vector.tensor_mul(attT[t][rs, qs], pn[hh][0:64, :], bc)

                # ---- partial Wo for this seq block ----
                for st in range(4):
                    stg = 4 * j + st
                    ss = slice(128 * stg, 128 * (stg + 1))
                    row0 = 128 * st if use_rs else 128 * stg
                    for ot in range(2):
                        os_ = slice(512 * ot, 512 * (ot + 1))
                        pw = psB.tile([128, 512], F32, tag="pw", name="pw")
                        for t in range(2):
                            nc.tensor.matmul(pw, attT[t][:, ss], wo_t[t][:, os_],
                                             start=(t == 0), stop=(t == 1))
                        ob = opool.tile([128, 512], F32, tag="ob", name="ob")
                        nc.vector.tensor_copy(ob, pw)
                        if use_rs:
                            nc.sync.dma_start(
                                out=cc_ins[j][row0:row0 + 128, os_], in_=ob)
                        else:
                            nc.sync.dma_start(out=out_d[ss, os_], in_=ob)
                if use_rs:
                    nc.gpsimd.collective_compute(
                        "ReduceScatter", mybir.AluOpType.add,
                        ins=[cc_ins[j][:, :]], outs=[cc_outs[j][:, :]],
                        replica_groups=groups)
                    nc.sync.dma_start(out=out_d[128 * j:128 * (j + 1), :],
                                      in_=cc_outs[j][:, :])

    nc.compile()
    return nc


def get_program(use_rs=False):
    key = bool(use_rs)
    if key not in _CACHED:
        _CACHED[key] = _build_program(key)
    return _CACHED[key]


def make_in_maps(x, Wq, Wk, Wv, Wo, token_positions):
    """Host-side sharding: per-core input dicts."""
    x = np.asarray(x, dtype=np.float32)
    Wq = np.asarray(Wq, dtype=np.float32)
    Wk = np.asarray(Wk, dtype=np.float32)
    Wv = np.asarray(Wv, dtype=np.float32)
    Wo = np.asarray(Wo, dtype=np.float32)
    pos = np.asarray(token_positions).astype(np.float32)

    # rope tables, feature-major [128, S]: row p -> pair index i = p % 32,
    # rows [0:32]=evens, [32:64]=odds per 64-row head block.
    i = np.arange(DH // 2, dtype=np.float32)
    d = THETA ** (2.0 * i / DH)                       # [32]
    tt = pos[None, :] / d[:, None]                    # [32, S]
    sin, cos = np.sin(tt), np.cos(tt)
    A = np.tile(cos, (4, 1)).astype(np.float32)       # [128, S]
    # B: evens row -> -sin, odds row -> +sin ; B2 = swap(B): evens->+sin, odds->-sin
    B2 = np.tile(np.concatenate([sin, -sin], axis=0), (2, 1)).astype(np.float32)

    # causal masks for the two diagonal pair-tiles [128, 1024] each:
    # pair0 halves r=0,128 ; pair1 halves r=256,384
    p = np.arange(128)[:, None]
    jj = np.arange(512)[None, :]
    def tri(r):
        return (jj >= p + r).astype(np.float32)
    maskP = np.concatenate(
        [np.concatenate([tri(0), tri(128)], axis=1),
         np.concatenate([tri(256), tri(384)], axis=1)], axis=0)  # [256, 1024]

    # per-head Q/K row permutation: evens then odds
    i2 = np.arange(DH // 2)
    perm = np.concatenate(
        [np.concatenate([64 * h + 2 * i2, 64 * h + 2 * i2 + 1]) for h in range(H)])

    in_maps = []
    for c in range(CORES):
        b, g = c // 4, c % 4
        rows = perm[F * g:F * (g + 1)]
        nat = np.arange(F * g, F * (g + 1))
        in_maps.append({
            "xT": np.ascontiguousarray(x[b].T),
            "wqT": np.ascontiguousarray(Wq[rows, :].T),
            "wkT": np.ascontiguousarray(Wk[rows, :].T),
            "wvT": np.ascontiguousarray(Wv[nat, :].T),
            "woT": np.ascontiguousarray(Wo[:, nat].T),
            "ropeA": A,
            "ropeB2": B2,
            "maskP": maskP,
        })
    return in_maps


def kernel(x, Wq, Wk, Wv, Wo, token_positions):
    from concourse.bass_utils import run_bass_kernel_spmd
    use_rs = bool(int(os.environ.get("MHA_USE_RS", "0")))
    nc = get_program(use_rs)
    in_maps = make_in_maps(x, Wq, Wk, Wv, Wo, token_positions)
    res = run_bass_kernel_spmd(nc, in_maps, list(range(CORES)))
    out = np.empty((B, S, D), dtype=np.float32)
    if use_rs:
        for b in range(B):
            for g in range(4):
                r = res.results[4 * b + g]["out_rs"]
                # out_rs rows: for each seq block j, rows 128j..128j+128
                # correspond to global rows 512*j + 128*g .. +128
                for j in range(NSB):
                    out[b, 512 * j + 128 * g:512 * j + 128 * (g + 1), :] = \
                        r[128 * j:128 * (j + 1), :]
    else:
        for b in range(B):
            acc = res.results[4 * b]["partial"].astype(np.float32).copy()
            for g in range(1, 4):
                acc += res.results[4 * b + g]["partial"]
            out[b] = acc
    return out
